# revision 11
# baseline (speedup 1.0000x reference)
"""Trainium2 Bass kernel for nn_Attentionv2 (B=8, N=1024, C=768, H=12, D=64).

Strategy: data-parallel over batch — one batch element per NeuronCore (8 cores).
Per core, multi-head attention is computed entirely in the "transposed"
orientation so no on-chip transposes are needed:

  QT[h*64+d, n] = sum_c WqT[c, h*64+d] * xT[c, n]     (head-pair tiles)
  KT likewise; V[n, h*64+d] = sum_c xT[c, n-tile] * WvT[c, :]
  ST[m, n]  = sum_d KT[d, m] * QT[d, n]               (scores transposed;
               the two heads of a pair sit on partitions 0-63 / 64-127 so
               their K=64 matmuls row-tile into the two PE array halves)
  ET        = exp(ST * 1/8)                            (no max-subtraction:
                                                        scores are O(1) here)
  PV lhsT   = [V_h | ones(64 cols)]  =>  out rows 0-63 = OT_h (unnorm),
               rows 64-127 = softmax denominator replicated 64x (free
               partition-broadcast done by the PE)
  OT_norm   = OT * (1/Z)                               (reciprocal via DVE,
                                                        straight from PSUM)
  y[n, o]   = sum_c OT_norm[c, n] * WpT[c, o] + bp[o]

Matmul operands are fp16 (full-rate PE, fast weight loads, HAM-warm clocks);
all accumulation is fp32 in PSUM.

v2 changes vs baseline (198us):
  - x is DMA'd FIRST across all three queues (sync/scalar/gpsimd) and the
    q/k weights are pair-major on the host so QK(pair 0) can start ~5us
    earlier; all input DMAs are fully contiguous per partition via the
    contraction-row remap c = 6p + o (both MM operands use the same map,
    so the accumulated contraction is unchanged).
  - softmax normalize drops the Z copy: reciprocal_approx_fast reads the
    denominator rows straight out of PSUM.
  - the output projection is split: pair 0-4 contributions run as PE
    filler inside the ACT-bound last-pair exp window (reusing the idle
    "qk" PSUM slots), with the bias folded into that add; only the pair-5
    rank-128 update + final add remain after the last normalize.
"""

import numpy as np

P = 128
B, N, C = 8, 1024, 768
H, D = 12, 64
SCALE = D ** -0.5  # 0.125
CT = C // P   # 6 contraction chunks
NT = N // P   # 8 sequence tiles
HP = H // 2   # 6 head pairs
NCORES = 8

_cache = {}


def _build_nc():
    import concourse.bass as bass
    import concourse.mybir as mybir
    import concourse.tile as tile
    from concourse import bacc

    f32 = mybir.dt.float32
    f16 = mybir.dt.float16
    Exp = mybir.ActivationFunctionType.Exp

    nc = bacc.Bacc("TRN2", target_bir_lowering=False, debug=False,
                   enable_asserts=False)

    xT = nc.dram_tensor("xT", [C, N], f16, kind="ExternalInput").ap()
    ident = nc.dram_tensor("ident", [P, P], f16, kind="ExternalInput").ap()
    wqT = nc.dram_tensor("wqT", [HP, C, 2 * D], f16, kind="ExternalInput").ap()
    wkT = nc.dram_tensor("wkT", [HP, C, 2 * D], f16, kind="ExternalInput").ap()
    wvT = nc.dram_tensor("wvT", [C, H * D], f16, kind="ExternalInput").ap()
    wpT = nc.dram_tensor("wpT", [C, C], f16, kind="ExternalInput").ap()
    bpb = nc.dram_tensor("bpb", [P, C], f32, kind="ExternalInput").ap()
    y = nc.dram_tensor("y", [N, C], f32, kind="ExternalOutput").ap()

    mm = nc.tensor.matmul

    # contraction-row remap: partition p of chunk o holds row c = 6p + o,
    # so every per-partition DMA run is contiguous in HBM.  x/wq/wk/wv all
    # use this map (both matmul operands see the same permutation); wp keeps
    # chunk==pair / partition==row-within-pair since its contraction runs
    # over the concat-head dim whose layout is fixed by ot.
    xTr = xT.rearrange("(p o) n -> p o n", o=CT)
    wqTr = wqT.rearrange("h (p o) f -> p h o f", o=CT)
    wkTr = wkT.rearrange("h (p o) f -> p h o f", o=CT)
    wvTr = wvT.rearrange("(p o) f -> p o f", o=CT)
    wpTr = wpT.rearrange("(o p) f -> p o f", p=P)

    with tile.TileContext(nc) as tc:
        with tc.tile_pool(name="persist", bufs=1) as persist:
            qt = persist.tile([P, HP, N], f16)        # QT: head pair j rows
            kt = persist.tile([P, HP, N], f16)
            vp = persist.tile([P, NT, H, 2 * D], f16)  # [Vh | ones]
            ot = persist.tile([P, HP, N], f16)        # normalized OT stacked
            wp_sb = persist.tile([P, CT, C], f16)
            bpb_sb = persist.tile([P, C], f32)

            with tc.tile_pool(name="ph1", bufs=1) as ph1, \
                 tc.tile_pool(name="mix", bufs=2, space="PSUM") as mix, \
                 tc.tile_pool(name="et", bufs=24) as etp, \
                 tc.tile_pool(name="sm", bufs=2) as smp, \
                 tc.tile_pool(name="ys", bufs=NT) as ysp_pool, \
                 tc.tile_pool(name="ys2", bufs=3) as ys2_pool, \
                 tc.tile_pool(name="ps_s", bufs=2, space="PSUM") as ps_s, \
                 tc.tile_pool(name="ps_o", bufs=2, space="PSUM") as ps_o:
                x_sb = ph1.tile([P, CT, N], f16)
                id_sb = ph1.tile([P, P], f16)
                wq_sb = ph1.tile([P, HP, CT, 2 * D], f16)
                wk_sb = ph1.tile([P, HP, CT, 2 * D], f16)
                wv_sb = ph1.tile([P, CT, H * D], f16)
                # Input DMAs: x first, chunk-interleaved on ALL three queues
                # (it gates every matmul; the c=0..2 chunks land first so the
                # QK(0) accumulation can begin early), then per-pair q/k
                # weight slices (each gates only its pair's QK), then v/p
                # weights + bias behind.
                nc.sync.dma_start(x_sb[:, 0:1, :], xTr[:, 0:1, :])
                nc.scalar.dma_start(x_sb[:, 1:2, :], xTr[:, 1:2, :])
                nc.gpsimd.dma_start(x_sb[:, 2:3, :], xTr[:, 2:3, :])
                nc.sync.dma_start(x_sb[:, 3:4, :], xTr[:, 3:4, :])
                nc.scalar.dma_start(x_sb[:, 4:5, :], xTr[:, 4:5, :])
                nc.gpsimd.dma_start(x_sb[:, 5:6, :], xTr[:, 5:6, :])
                for j in range(HP):
                    nc.sync.dma_start(wq_sb[:, j], wqTr[:, j])
                    nc.scalar.dma_start(wk_sb[:, j], wkTr[:, j])
                nc.gpsimd.dma_start(id_sb[:], ident)
                nc.gpsimd.dma_start(wv_sb[:], wvTr[:])
                nc.gpsimd.dma_start(wp_sb[:], wpTr[:])
                nc.scalar.dma_start(bpb_sb[:], bpb)

                def at(us):
                    # sim-time floor: places instructions in the static
                    # per-engine order without affecting runtime waits
                    return tc.tile_wait_until(us / 1000.0)

                # scratch + exp-table preload + HAM warmup matmuls: keep the
                # PE clock warm across the input-DMA window (x lands ~13us).
                scr = ph1.tile([P, 512], f16)
                scrt = ph1.tile([P, 16], f32)
                nc.vector.memset(scr[:], 0.01)
                nc.vector.memset(vp[:, :, :, D:2 * D], 1.0)
                nc.scalar.activation(scrt[:], scr[:, 0:16], Exp, scale=1.0)
                wps = ps_o.tile([P, 512], f32, tag="o", name="wps")
                for i in range(8):
                    mm(wps[:], lhsT=scr[:, 0:128], rhs=scr[:],
                       start=True, stop=True)
                with at(8.5):
                    for i in range(6):
                        mm(wps[:], lhsT=scr[:, 0:128], rhs=scr[:],
                           start=True, stop=True)
                with at(10.5):
                    for i in range(6):
                        mm(wps[:], lhsT=scr[:, 0:128], rhs=scr[:],
                           start=True, stop=True)
                with at(12.2):
                    for i in range(4):
                        mm(wps[:], lhsT=scr[:, 0:128], rhs=scr[:],
                           start=True, stop=True)

                def emit_qk(j):
                    for w_sb, dst in ((wq_sb, qt), (wk_sb, kt)):
                        for nh in range(2):
                            ps = mix.tile([P, 512], f32, tag="qk",
                                          name="qkps")
                            for c in range(CT):
                                mm(ps[:], lhsT=w_sb[:, j, c, :],
                                   rhs=x_sb[:, c, nh * 512:(nh + 1) * 512],
                                   start=(c == 0), stop=(c == CT - 1))
                            nc.vector.tensor_copy(
                                dst[:, j, nh * 512:(nh + 1) * 512], ps[:])

                emit_qk(0)

                for t in range(NT):
                    psa = mix.tile([P, 512], f32, tag="qk", name="psa")
                    psb = mix.tile([P, 512], f32, tag="qk", name="psb")
                    for c in range(CT):
                        lh = x_sb[:, c, t * P:(t + 1) * P]
                        mm(psa[:], lhsT=lh, rhs=wv_sb[:, c, 0:512],
                           start=(c == 0), stop=(c == CT - 1))
                        mm(psb[:, 0:256], lhsT=lh, rhs=wv_sb[:, c, 512:768],
                           start=(c == 0), stop=(c == CT - 1))
                    nc.vector.tensor_copy(
                        vp[:, t, 0:8, 0:D],
                        psa.rearrange("p (h d) -> p h d", d=D))
                    nc.vector.tensor_copy(
                        vp[:, t, 8:12, 0:D],
                        psb[:, 0:256].rearrange("p (h d) -> p h d", d=D))

                ets = {}

                def emit_scores_mt(j, mt):
                    # Both heads' scores for one nh-half share one PSUM
                    # tile, so each exp depends on both row-group matmuls
                    # and the scheduler cannot split the dual-stream pair.
                    s = {}
                    for nh in range(2):
                        s[nh] = ps_s.tile([P, 2, 512], f32, tag="s",
                                          name=f"s_{nh}")
                        ets[(j, mt, nh)] = etp.tile([P, 2, 512], f16,
                                                    tag="et", name=f"et_{nh}")
                    for nh in range(2):
                        for hh in range(2):   # adjacent => PE row-tiling
                            r0 = hh * D
                            mm(s[nh][:, hh, :],
                               lhsT=kt[r0:r0 + D, j, mt * P:(mt + 1) * P],
                               rhs=qt[r0:r0 + D, j, nh * 512:(nh + 1) * 512],
                               start=True, stop=True)
                    for nh in range(2):
                        nc.scalar.activation(ets[(j, mt, nh)][:], s[nh][:],
                                             Exp, scale=float(SCALE))

                def emit_pv_norm(j):
                    for hh in range(2):
                        h = 2 * j + hh
                        r0 = hh * D
                        pso = {nh: ps_o.tile([P, 512], f32, tag="o",
                                             name=f"o_{nh}")
                               for nh in range(2)}
                        for mt in range(NT):   # dense 16-MM PV burst
                            for nh in range(2):
                                mm(pso[nh][:],
                                   lhsT=vp[:, mt, h],
                                   rhs=ets[(j, mt, nh)][:, hh, :],
                                   start=(mt == 0), stop=(mt == NT - 1))
                        for nh in range(2):
                            sums = smp.tile([D, 512], f32, tag="sums")
                            rec = smp.tile([D, 512], f32, tag="rec")
                            nc.vector.tensor_copy(sums[:],
                                                  pso[nh][D:2 * D, :])
                            nc.vector.reciprocal_approx_fast(rec[:], sums[:])
                            nc.vector.tensor_mul(
                                ot[r0:r0 + D, j, nh * 512:(nh + 1) * 512],
                                pso[nh][0:D, :], rec[:])

                def emit_pv_norm_last(j):
                    # last pair: scores are done, so the ps_s banks are free.
                    # One [P, 2, 512] tile per hh holds both nh halves, so
                    # hh=1's PV runs while hh=0's normalize drains (no ps_o
                    # double-buffer stall), and the normalize runs FD=1024.
                    for hh in range(2):
                        h = 2 * j + hh
                        r0 = hh * D
                        pso = ps_s.tile([P, 2, 512], f32, tag="s",
                                        name=f"ol_{hh}")
                        for mt in range(NT):
                            for nh in range(2):
                                mm(pso[:, nh, :],
                                   lhsT=vp[:, mt, h],
                                   rhs=ets[(j, mt, nh)][:, hh, :],
                                   start=(mt == 0), stop=(mt == NT - 1))
                        sums = smp.tile([D, 2, 512], f32, tag="sums2")
                        rec = smp.tile([D, 2, 512], f32, tag="rec2")
                        nc.vector.tensor_copy(sums[:], pso[D:2 * D, :, :])
                        nc.vector.reciprocal_approx_fast(rec[:], sums[:])
                        nc.vector.tensor_mul(
                            ot[r0:r0 + D, j, :], pso[0:D, :, :], rec[:])

                yre = y.rearrange("(t p) f -> t p f", p=P)
                ys = {}

                def emit_outproj_a():
                    # pair 0-4 contributions of the output projection: PE
                    # filler for the ACT-bound last-pair exp window.  Bias
                    # is folded into the evacuation add.
                    for t in range(NT):
                        pa = mix.tile([P, 512], f32, tag="qk", name="ya")
                        pb = mix.tile([P, 512], f32, tag="qk", name="yb")
                        for c in range(HP - 1):
                            lh = ot[:, c, t * P:(t + 1) * P]
                            mm(pa[:], lhsT=lh, rhs=wp_sb[:, c, 0:512],
                               start=(c == 0), stop=(c == HP - 2))
                            mm(pb[:, 0:256], lhsT=lh,
                               rhs=wp_sb[:, c, 512:768],
                               start=(c == 0), stop=(c == HP - 2))
                        yt = ysp_pool.tile([P, C], f16, tag="ys")
                        ys[t] = yt
                        nc.vector.tensor_add(yt[:, 0:512], pa[:],
                                             bpb_sb[:, 0:512])
                        nc.vector.tensor_add(yt[:, 512:768], pb[:, 0:256],
                                             bpb_sb[:, 512:768])

                # software-pipelined: PV/normalize of pair j-1 lands after
                # pair j's first score steps so ACT never stalls at pair
                # boundaries; QK of pair j+1 fills mid-pair PE gaps; the
                # pair 0-4 output projection fills the last pair's window.
                for j in range(HP):
                    for mt in range(NT):
                        emit_scores_mt(j, mt)
                        if mt == 1:
                            if j > 0:
                                emit_pv_norm(j - 1)
                            if j + 1 < HP:
                                emit_qk(j + 1)
                            if j == HP - 1:
                                emit_outproj_a()
                emit_pv_norm_last(HP - 1)

                # tail: pair-5 rank-128 update; the phase-A partial ys[t] is
                # folded in via an identity matmul so the evacuation is a
                # plain PSUM->SBUF copy, split across the now-idle ACT and
                # DVE engines (no DVE-bound add chain at the very end).
                for t in range(NT):
                    pa = mix.tile([P, 512], f32, tag="qk", name="ya2")
                    pb = mix.tile([P, 512], f32, tag="qk", name="yb2")
                    lh = ot[:, HP - 1, t * P:(t + 1) * P]
                    mm(pa[:], lhsT=lh, rhs=wp_sb[:, HP - 1, 0:512],
                       start=True, stop=False)
                    mm(pa[:], lhsT=id_sb[:], rhs=ys[t][:, 0:512],
                       start=False, stop=True)
                    mm(pb[:, 0:256], lhsT=lh, rhs=wp_sb[:, HP - 1, 512:768],
                       start=True, stop=False)
                    mm(pb[:, 0:256], lhsT=id_sb[:], rhs=ys[t][:, 512:768],
                       start=False, stop=True)
                    y2 = ys2_pool.tile([P, C], f32, tag="ys2")
                    if t % 2 == 0:
                        nc.scalar.copy(y2[:, 0:512], pa[:])
                        nc.scalar.copy(y2[:, 512:768], pb[:, 0:256])
                    else:
                        nc.vector.tensor_copy(y2[:, 0:512], pa[:])
                        nc.vector.tensor_copy(y2[:, 512:768], pb[:, 0:256])
                    eng = nc.sync if t % 2 == 0 else nc.scalar
                    eng.dma_start(yre[t], y2[:])

    nc.compile()
    return nc


def _get_nc():
    if "nc" not in _cache:
        _cache["nc"] = _build_nc()
    return _cache["nc"]


def _make_in_maps(x, Wq, Wk, Wv, Wp, bp):
    x = np.asarray(x, dtype=np.float32)
    # pair-major q/k weights: [HP, C, 2D] so each pair's slice is one
    # contiguous DMA
    wq = np.asarray(Wq, np.float32).reshape(HP, 2 * D, C)
    wk = np.asarray(Wk, np.float32).reshape(HP, 2 * D, C)
    wqT = np.ascontiguousarray(wq.transpose(0, 2, 1).astype(np.float16))
    wkT = np.ascontiguousarray(wk.transpose(0, 2, 1).astype(np.float16))
    wvT = np.ascontiguousarray(
        np.asarray(Wv, np.float32).reshape(H * D, C).T.astype(np.float16))
    wpT = np.ascontiguousarray(
        np.asarray(Wp, np.float32).T.astype(np.float16))
    bpb = np.ascontiguousarray(
        np.broadcast_to(np.asarray(bp, np.float32), (P, C)))
    ident = np.ascontiguousarray(np.eye(P, dtype=np.float16))
    in_maps = []
    for b in range(NCORES):
        in_maps.append({
            "xT": np.ascontiguousarray(x[b].T.astype(np.float16)),
            "wqT": wqT, "wkT": wkT, "wvT": wvT, "wpT": wpT, "bpb": bpb,
            "ident": ident,
        })
    return in_maps


def run(x, Wq, Wk, Wv, Wp, bp, trace=False):
    from concourse.bass_utils import run_bass_kernel_spmd
    nc = _get_nc()
    in_maps = _make_in_maps(x, Wq, Wk, Wv, Wp, bp)
    res = run_bass_kernel_spmd(nc, in_maps, list(range(NCORES)), trace=trace)
    out = np.stack([res.results[b]["y"] for b in range(NCORES)])
    return out, res


def kernel(x, Wq, Wk, Wv, Wp, bp):
    out, _ = run(x, Wq, Wk, Wv, Wp, bp)
    return out


# revision 16
# speedup vs baseline: 1.0208x; 1.0208x over previous
"""Trainium2 Bass kernel for nn_Attentionv2 (B=8, N=1024, C=768, H=12, D=64).

Strategy: data-parallel over batch — one batch element per NeuronCore (8 cores).
Per core, multi-head attention is computed entirely in the "transposed"
orientation so no on-chip transposes are needed:

  QT[h*64+d, n] = sum_c WqT[c, h*64+d] * xT[c, n]     (head-pair tiles)
  KT likewise; V[n, h*64+d] = sum_c xT[c, n-tile] * WvT[c, :]
  ST[m, n]  = sum_d KT[d, m] * QT[d, n]               (scores transposed;
               the two heads of a pair sit on partitions 0-63 / 64-127 so
               their K=64 matmuls row-tile into the two PE array halves)
  ET        = exp(ST * 1/8)                            (no max-subtraction:
                                                        scores are O(1) here)
  PV lhsT   = [V_h | ones(64 cols)]  =>  out rows 0-63 = OT_h (unnorm),
               rows 64-127 = softmax denominator replicated 64x (free
               partition-broadcast done by the PE)
  OT_norm   = OT * (1/Z)                               (reciprocal via DVE,
                                                        straight from PSUM)
  y[n, o]   = sum_c OT_norm[c, n] * WpT[c, o] + bp[o]

Matmul operands are fp16 (full-rate PE, fast weight loads, HAM-warm clocks);
all accumulation is fp32 in PSUM.

v2 changes vs baseline (198us):
  - x is DMA'd FIRST across all three queues (sync/scalar/gpsimd) and the
    q/k weights are pair-major on the host so QK(pair 0) can start ~5us
    earlier; all input DMAs are fully contiguous per partition via the
    contraction-row remap c = 6p + o (both MM operands use the same map,
    so the accumulated contraction is unchanged).
  - softmax normalize drops the Z copy: reciprocal_approx_fast reads the
    denominator rows straight out of PSUM.
  - the output projection is split: pair 0-4 contributions run as PE
    filler inside the ACT-bound last-pair exp window (reusing the idle
    "qk" PSUM slots), with the bias folded into that add; only the pair-5
    rank-128 update + final add remain after the last normalize.
"""

import numpy as np

P = 128
B, N, C = 8, 1024, 768
H, D = 12, 64
SCALE = D ** -0.5  # 0.125
CT = C // P   # 6 contraction chunks
NT = N // P   # 8 sequence tiles
HP = H // 2   # 6 head pairs
NCORES = 8

_cache = {}


def _build_nc():
    import concourse.bass as bass
    import concourse.mybir as mybir
    import concourse.tile as tile
    from concourse import bacc

    f32 = mybir.dt.float32
    f16 = mybir.dt.float16
    Exp = mybir.ActivationFunctionType.Exp

    nc = bacc.Bacc("TRN2", target_bir_lowering=False, debug=False,
                   enable_asserts=False)

    xT = nc.dram_tensor("xT", [C, N], f16, kind="ExternalInput").ap()
    ident = nc.dram_tensor("ident", [P, P], f16, kind="ExternalInput").ap()
    wqT = nc.dram_tensor("wqT", [HP, C, 2 * D], f16, kind="ExternalInput").ap()
    wkT = nc.dram_tensor("wkT", [HP, C, 2 * D], f16, kind="ExternalInput").ap()
    wvT = nc.dram_tensor("wvT", [C, H * D], f16, kind="ExternalInput").ap()
    wpT = nc.dram_tensor("wpT", [C, C], f16, kind="ExternalInput").ap()
    bpb = nc.dram_tensor("bpb", [P, C], f32, kind="ExternalInput").ap()
    y = nc.dram_tensor("y", [N, C], f32, kind="ExternalOutput").ap()

    mm = nc.tensor.matmul

    # contraction-row remap: partition p of chunk o holds row c = 6p + o,
    # so every per-partition DMA run is contiguous in HBM.  x/wq/wk/wv all
    # use this map (both matmul operands see the same permutation); wp keeps
    # chunk==pair / partition==row-within-pair since its contraction runs
    # over the concat-head dim whose layout is fixed by ot.
    xTr = xT.rearrange("(p o) n -> p o n", o=CT)
    wqTr = wqT.rearrange("h (p o) f -> p h o f", o=CT)
    wkTr = wkT.rearrange("h (p o) f -> p h o f", o=CT)
    wvTr = wvT.rearrange("(p o) f -> p o f", o=CT)
    wpTr = wpT.rearrange("(o p) f -> p o f", p=P)

    with tile.TileContext(nc) as tc:
        with tc.tile_pool(name="persist", bufs=1) as persist:
            qt = persist.tile([P, HP, N], f16)        # QT: head pair j rows
            kt = persist.tile([P, HP, N], f16)
            vp = persist.tile([P, NT, H, 2 * D], f16)  # [Vh | ones]
            ot = persist.tile([P, HP, N], f16)        # normalized OT stacked
            wp_sb = persist.tile([P, CT, C], f16)
            bpb_sb = persist.tile([P, C], f32)

            with tc.tile_pool(name="ph1", bufs=1) as ph1, \
                 tc.tile_pool(name="mix", bufs=2, space="PSUM") as mix, \
                 tc.tile_pool(name="et", bufs=24) as etp, \
                 tc.tile_pool(name="sm", bufs=2) as smp, \
                 tc.tile_pool(name="ys", bufs=NT) as ysp_pool, \
                 tc.tile_pool(name="ys2", bufs=3) as ys2_pool, \
                 tc.tile_pool(name="ps_s", bufs=2, space="PSUM") as ps_s, \
                 tc.tile_pool(name="ps_o", bufs=2, space="PSUM") as ps_o:
                x_sb = ph1.tile([P, CT, N], f16)
                id_sb = ph1.tile([P, P], f16)
                wq_sb = ph1.tile([P, HP, CT, 2 * D], f16)
                wk_sb = ph1.tile([P, HP, CT, 2 * D], f16)
                wv_sb = ph1.tile([P, CT, H * D], f16)
                # Input DMAs: x first, chunk-interleaved on ALL three queues
                # (it gates every matmul; the c=0..2 chunks land first so the
                # QK(0) accumulation can begin early), then per-pair q/k
                # weight slices (each gates only its pair's QK), then v/p
                # weights + bias behind.
                nc.sync.dma_start(x_sb[:, 0:1, :], xTr[:, 0:1, :])
                nc.scalar.dma_start(x_sb[:, 1:2, :], xTr[:, 1:2, :])
                nc.gpsimd.dma_start(x_sb[:, 4:6, :], xTr[:, 4:6, :])
                nc.sync.dma_start(wq_sb[:, 0], wqTr[:, 0])
                nc.scalar.dma_start(wk_sb[:, 0], wkTr[:, 0])
                nc.sync.dma_start(x_sb[:, 2:3, :], xTr[:, 2:3, :])
                nc.scalar.dma_start(x_sb[:, 3:4, :], xTr[:, 3:4, :])
                for j in range(1, HP):
                    nc.sync.dma_start(wq_sb[:, j], wqTr[:, j])
                    nc.scalar.dma_start(wk_sb[:, j], wkTr[:, j])
                nc.gpsimd.dma_start(id_sb[:], ident)
                nc.gpsimd.dma_start(wv_sb[:], wvTr[:])
                nc.gpsimd.dma_start(wp_sb[:], wpTr[:])
                nc.scalar.dma_start(bpb_sb[:], bpb)

                def at(us):
                    # sim-time floor: places instructions in the static
                    # per-engine order without affecting runtime waits
                    return tc.tile_wait_until(us / 1000.0)

                # scratch + exp-table preload + HAM warmup matmuls: keep the
                # PE clock warm across the input-DMA window (x lands ~13us).
                scr = ph1.tile([P, 512], f16)
                scrt = ph1.tile([P, 16], f32)
                nc.vector.memset(scr[:], 0.01)
                nc.vector.memset(vp[:, :, :, D:2 * D], 1.0)
                nc.scalar.activation(scrt[:], scr[:, 0:16], Exp, scale=1.0)
                wps = ps_o.tile([P, 512], f32, tag="o", name="wps")
                for i in range(8):
                    mm(wps[:], lhsT=scr[:, 0:128], rhs=scr[:],
                       start=True, stop=True)
                with at(8.5):
                    for i in range(6):
                        mm(wps[:], lhsT=scr[:, 0:128], rhs=scr[:],
                           start=True, stop=True)
                with at(10.5):
                    for i in range(6):
                        mm(wps[:], lhsT=scr[:, 0:128], rhs=scr[:],
                           start=True, stop=True)
                with at(12.2):
                    for i in range(4):
                        mm(wps[:], lhsT=scr[:, 0:128], rhs=scr[:],
                           start=True, stop=True)

                def emit_qk(j):
                    for w_sb, dst in ((wq_sb, qt), (wk_sb, kt)):
                        for nh in range(2):
                            ps = mix.tile([P, 512], f32, tag="qk",
                                          name="qkps")
                            for c in range(CT):
                                mm(ps[:], lhsT=w_sb[:, j, c, :],
                                   rhs=x_sb[:, c, nh * 512:(nh + 1) * 512],
                                   start=(c == 0), stop=(c == CT - 1))
                            nc.vector.tensor_copy(
                                dst[:, j, nh * 512:(nh + 1) * 512], ps[:])

                emit_qk(0)

                for t in range(NT):
                    psa = mix.tile([P, 512], f32, tag="qk", name="psa")
                    psb = mix.tile([P, 512], f32, tag="qk", name="psb")
                    for c in range(CT):
                        lh = x_sb[:, c, t * P:(t + 1) * P]
                        mm(psa[:], lhsT=lh, rhs=wv_sb[:, c, 0:512],
                           start=(c == 0), stop=(c == CT - 1))
                        mm(psb[:, 0:256], lhsT=lh, rhs=wv_sb[:, c, 512:768],
                           start=(c == 0), stop=(c == CT - 1))
                    nc.vector.tensor_copy(
                        vp[:, t, 0:8, 0:D],
                        psa.rearrange("p (h d) -> p h d", d=D))
                    nc.vector.tensor_copy(
                        vp[:, t, 8:12, 0:D],
                        psb[:, 0:256].rearrange("p (h d) -> p h d", d=D))

                ets = {}

                def emit_scores_mt(j, mt):
                    # Both heads' scores for one nh-half share one PSUM
                    # tile, so each exp depends on both row-group matmuls
                    # and the scheduler cannot split the dual-stream pair.
                    s = {}
                    for nh in range(2):
                        s[nh] = ps_s.tile([P, 2, 512], f32, tag="s",
                                          name=f"s_{nh}")
                        ets[(j, mt, nh)] = etp.tile([P, 2, 512], f16,
                                                    tag="et", name=f"et_{nh}")
                    for nh in range(2):
                        for hh in range(2):   # adjacent => PE row-tiling
                            r0 = hh * D
                            mm(s[nh][:, hh, :],
                               lhsT=kt[r0:r0 + D, j, mt * P:(mt + 1) * P],
                               rhs=qt[r0:r0 + D, j, nh * 512:(nh + 1) * 512],
                               start=True, stop=True)
                    for nh in range(2):
                        nc.scalar.activation(ets[(j, mt, nh)][:], s[nh][:],
                                             Exp, scale=float(SCALE))

                def emit_pv_norm(j):
                    for hh in range(2):
                        h = 2 * j + hh
                        r0 = hh * D
                        pso = {nh: ps_o.tile([P, 512], f32, tag="o",
                                             name=f"o_{nh}")
                               for nh in range(2)}
                        for mt in range(NT):   # dense 16-MM PV burst
                            for nh in range(2):
                                mm(pso[nh][:],
                                   lhsT=vp[:, mt, h],
                                   rhs=ets[(j, mt, nh)][:, hh, :],
                                   start=(mt == 0), stop=(mt == NT - 1))
                        for nh in range(2):
                            sums = smp.tile([D, 512], f32, tag="sums")
                            rec = smp.tile([D, 512], f32, tag="rec")
                            nc.vector.tensor_copy(sums[:],
                                                  pso[nh][D:2 * D, :])
                            nc.vector.reciprocal_approx_fast(rec[:], sums[:])
                            nc.vector.tensor_mul(
                                ot[r0:r0 + D, j, nh * 512:(nh + 1) * 512],
                                pso[nh][0:D, :], rec[:])

                def emit_pv_norm_last(j):
                    # last pair: hh=0 accumulates in the early-freed ps_o
                    # slots (its MMs run as the last exps land); hh=1 goes
                    # through one [P,2,512] ps_s tile, whose slot frees at
                    # the last exp — so hh=1's PV overlaps hh=0's normalize.
                    # Z evacuation copies run on the now-idle ACT so DVE
                    # only does recip+mul.
                    pso0 = {nh: ps_o.tile([P, 512], f32, tag="o",
                                          name=f"ol0_{nh}")
                            for nh in range(2)}
                    for mt in range(NT):
                        for nh in range(2):
                            mm(pso0[nh][:],
                               lhsT=vp[:, mt, 2 * j],
                               rhs=ets[(j, mt, nh)][:, 0, :],
                               start=(mt == 0), stop=(mt == NT - 1))
                    pso1 = ps_s.tile([P, 2, 512], f32, tag="s", name="ol1")
                    for mt in range(NT):
                        for nh in range(2):
                            mm(pso1[:, nh, :],
                               lhsT=vp[:, mt, 2 * j + 1],
                               rhs=ets[(j, mt, nh)][:, 1, :],
                               start=(mt == 0), stop=(mt == NT - 1))
                    for nh in range(2):
                        sums = smp.tile([D, 512], f32, tag="sums")
                        rec = smp.tile([D, 512], f32, tag="rec")
                        nc.scalar.copy(sums[:], pso0[nh][D:2 * D, :])
                        nc.vector.reciprocal_approx_fast(rec[:], sums[:])
                        nc.vector.tensor_mul(
                            ot[0:D, j, nh * 512:(nh + 1) * 512],
                            pso0[nh][0:D, :], rec[:])
                    sums1 = smp.tile([D, 2, 512], f32, tag="sums2")
                    rec1 = smp.tile([D, 2, 512], f32, tag="rec2")
                    nc.scalar.copy(sums1[:], pso1[D:2 * D, :, :])
                    nc.vector.reciprocal_approx_fast(rec1[:], sums1[:])
                    nc.vector.tensor_mul(
                        ot[D:2 * D, j, :], pso1[0:D, :, :], rec1[:])

                yre = y.rearrange("(t p) f -> t p f", p=P)
                ys = {}

                def emit_outproj_a(ts):
                    # pair 0-4 contributions of the output projection: PE
                    # filler for the ACT-bound last-pair exp window.  Bias
                    # is folded into the evacuation add.
                    for t in ts:
                        pa = mix.tile([P, 512], f32, tag="qk", name="ya")
                        pb = mix.tile([P, 512], f32, tag="qk", name="yb")
                        for c in range(HP - 1):
                            lh = ot[:, c, t * P:(t + 1) * P]
                            mm(pa[:], lhsT=lh, rhs=wp_sb[:, c, 0:512],
                               start=(c == 0), stop=(c == HP - 2))
                            mm(pb[:, 0:256], lhsT=lh,
                               rhs=wp_sb[:, c, 512:768],
                               start=(c == 0), stop=(c == HP - 2))
                        yt = ysp_pool.tile([P, C], f16, tag="ys")
                        ys[t] = yt
                        nc.vector.tensor_add(yt[:, 0:512], pa[:],
                                             bpb_sb[:, 0:512])
                        nc.vector.tensor_add(yt[:, 512:768], pb[:, 0:256],
                                             bpb_sb[:, 512:768])

                # software-pipelined: PV/normalize of pair j-1 lands after
                # pair j's first score steps so ACT never stalls at pair
                # boundaries; QK of pair j+1 fills mid-pair PE gaps; the
                # pair 0-4 output projection fills the last pair's window.
                # software-pipelined: QK(j+1) is emitted BEFORE PV(j-1) so
                # its qt/kt evacuation copies land early in the DVE order —
                # otherwise the next pair's first scores get statically
                # scheduled behind the whole PV+normalize block and the exp
                # chain stalls ~3.5us at every pair boundary.
                for j in range(HP):
                    for mt in range(NT):
                        emit_scores_mt(j, mt)
                        if mt == 1:
                            if j + 1 < HP:
                                emit_qk(j + 1)
                            if j > 0:
                                emit_pv_norm(j - 1)
                            if j == HP - 1:
                                emit_outproj_a(range(6))
                emit_pv_norm_last(HP - 1)
                emit_outproj_a(range(6, NT))

                # tail: pair-5 rank-128 update; the phase-A partial ys[t] is
                # folded in via an identity matmul so the evacuation is a
                # plain PSUM->SBUF copy, split across the now-idle ACT and
                # DVE engines (no DVE-bound add chain at the very end).
                for t in range(NT):
                    pa = mix.tile([P, 512], f32, tag="qk", name="ya2")
                    pb = mix.tile([P, 512], f32, tag="qk", name="yb2")
                    lh = ot[:, HP - 1, t * P:(t + 1) * P]
                    mm(pa[:], lhsT=lh, rhs=wp_sb[:, HP - 1, 0:512],
                       start=True, stop=False)
                    mm(pa[:], lhsT=id_sb[:], rhs=ys[t][:, 0:512],
                       start=False, stop=True)
                    mm(pb[:, 0:256], lhsT=lh, rhs=wp_sb[:, HP - 1, 512:768],
                       start=True, stop=False)
                    mm(pb[:, 0:256], lhsT=id_sb[:], rhs=ys[t][:, 512:768],
                       start=False, stop=True)
                    y2 = ys2_pool.tile([P, C], f32, tag="ys2")
                    if t % 2 == 0:
                        nc.scalar.copy(y2[:, 0:512], pa[:])
                        nc.scalar.copy(y2[:, 512:768], pb[:, 0:256])
                    else:
                        nc.vector.tensor_copy(y2[:, 0:512], pa[:])
                        nc.vector.tensor_copy(y2[:, 512:768], pb[:, 0:256])
                    eng = nc.sync if t % 2 == 0 else nc.scalar
                    eng.dma_start(yre[t], y2[:])

    nc.compile()
    return nc


def _get_nc():
    if "nc" not in _cache:
        _cache["nc"] = _build_nc()
    return _cache["nc"]


def _make_in_maps(x, Wq, Wk, Wv, Wp, bp):
    x = np.asarray(x, dtype=np.float32)
    # pair-major q/k weights: [HP, C, 2D] so each pair's slice is one
    # contiguous DMA
    wq = np.asarray(Wq, np.float32).reshape(HP, 2 * D, C)
    wk = np.asarray(Wk, np.float32).reshape(HP, 2 * D, C)
    wqT = np.ascontiguousarray(wq.transpose(0, 2, 1).astype(np.float16))
    wkT = np.ascontiguousarray(wk.transpose(0, 2, 1).astype(np.float16))
    wvT = np.ascontiguousarray(
        np.asarray(Wv, np.float32).reshape(H * D, C).T.astype(np.float16))
    wpT = np.ascontiguousarray(
        np.asarray(Wp, np.float32).T.astype(np.float16))
    bpb = np.ascontiguousarray(
        np.broadcast_to(np.asarray(bp, np.float32), (P, C)))
    ident = np.ascontiguousarray(np.eye(P, dtype=np.float16))
    in_maps = []
    for b in range(NCORES):
        in_maps.append({
            "xT": np.ascontiguousarray(x[b].T.astype(np.float16)),
            "wqT": wqT, "wkT": wkT, "wvT": wvT, "wpT": wpT, "bpb": bpb,
            "ident": ident,
        })
    return in_maps


def run(x, Wq, Wk, Wv, Wp, bp, trace=False):
    from concourse.bass_utils import run_bass_kernel_spmd
    nc = _get_nc()
    in_maps = _make_in_maps(x, Wq, Wk, Wv, Wp, bp)
    res = run_bass_kernel_spmd(nc, in_maps, list(range(NCORES)), trace=trace)
    out = np.stack([res.results[b]["y"] for b in range(NCORES)])
    return out, res


def kernel(x, Wq, Wk, Wv, Wp, bp):
    out, _ = run(x, Wq, Wk, Wv, Wp, bp)
    return out
